# revision 9
# baseline (speedup 1.0000x reference)
"""AxonLIFNode forward on 8 Trainium2 NeuronCores.

Reference recurrence (per element, sequential over T):
    mem   = mem + (x_t + V_RESET - mem) / TAU        # V_RESET=0, TAU=2
    spike = (mem - V_TH > 0)                         # V_TH=1, {0.0, 1.0}
    mem   = (1 - spike) * mem + V_RESET * spike      # reset to 0 on spike
    out_i = out_i * sigmoid(w) + spike               # axon current (w=0 -> 0.5)
    outputs: (spike, out_i), both [B, T, N] f32

Strategy: data-parallel over the batch axis (B=64 -> 8 per core). Per core the
32768 independent series are laid out as 128 partitions x 256 free elements;
both recurrences live in linear [P, T+1, F] SBUF buffers (slot 0 = zero init)
and advance MULTIPLE timesteps per DVE instruction via within-instruction
RAW: the op's out stream writes slot t+1 while its state input reads slot t,
i.e. the same SBUF address the instruction itself wrote F=256 elements (~256
DVE cycles, far beyond pipeline depth) earlier. Per group of M timesteps:

    m1 : prev = m1 * (m1 <= 1); m1' = prev + (x - prev) * 0.5   (f32, exact)
    oi : oi' = oi * inv_tau + (m1' > 1)                         (fp16 state)

m1 is bit-exact vs. the reference ordering (each ALU stage is one IEEE f32
rounding; *0.5 == /2 exactly). oi is stored and streamed out as fp16: the
per-step rounding error decays by inv_tau so the accumulated relative error
stays ~2^-10, far inside the harness gate, and the HBM write traffic halves.
Spikes are produced off the critical path on the Scalar(ACT) engine with a
saturated sigmoid (exact {0,1}, see _build) and stored as fp8 (lossless).
X streams in on the SP HWDGE ring (issued up front, batch per group); spike
and oi groups stream out on the ACT ring as each group completes. Group
sizes taper ([4,4,16,...]) so compute starts early and the tail drains small.
"""

import numpy as np

import concourse.bacc as bacc
import concourse.mybir as mybir
import concourse.dve_ops as dve_ops
from concourse.dve_ops import DveOp
from concourse.dve_spec import Spec, Src0, Src1, C0, C1, lower
from concourse.dve_uop import DveOpSpec
from concourse.tile import TileContext
from concourse.bass_utils import run_bass_kernel_spmd

# Problem shape (hardcoded per harness contract).
B, T, N = 64, 64, 4096
CORES = 8
BS = B // CORES          # batches per core
P = 128                  # SBUF partitions
J = 16                   # n-chunks per batch: BS * J == P
F = N // J               # free elements per partition per timestep (256)
GROUPS = [4, 4, 16, 16, 16, 4, 4]   # timesteps per DVE instruction / DMA group
SPK_SCALE = 1.0e9        # sigmoid saturation trick scale (see _build)

def _register_op(name: str, spec: Spec) -> DveOp:
    """Register a custom DVE op in the global registry with a computed sha."""
    for op in dve_ops.OPS:
        if op.name == name:
            return op
    row = dve_ops._CUSTOM_DVE_ROW_BASE + len(dve_ops.OPS)
    assert row < 0x20, "custom-DVE opcode rows exhausted"
    shas = {}
    for ver in ("v3", "v4"):
        uops = lower(spec, ver=ver)
        shas[ver] = DveOpSpec(name=name, opcode=row, uops=uops, rd1_en=True).sha(ver)
    op = DveOp(name, spec, subdim=False, uops_sha=shas)
    dve_ops._SUB_OPCODE_FOR_NAME[name] = row
    dve_ops.OPS.append(op)
    dve_ops.CUSTOM_DVE_SPECS[name] = spec
    return op


def _lif_ops() -> tuple[DveOp, DveOp]:
    """LIF_M1: m1_t from (x_t, m1_{t-1}); LIF_OI: oi_t from (oi_{t-1}, m1_t).

    LIF_M1: out = prev + (Src0 - prev) * C0, prev = Src1 * (Src1 <= C1)
    LIF_OI: out = Src0 * C0 + (Src1 > C1)
    Each ALU stage is one IEEE f32 rounding; bit-exact vs the reference.
    """
    keep = Src1 <= C1
    prev = Src1 * keep
    m1 = _register_op(
        "LIF_M1_ANT",
        Spec(
            body=prev + (Src0 - prev) * C0,
            reference=lambda in0, in1, s0, s1, imm2: (
                (p := (in1 * (in1 <= s1)).astype(np.float32))
                + (in0 - p) * np.float32(s0)
            ).astype(np.float32),
        ),
    )
    oi = _register_op(
        "LIF_OI_ANT",
        Spec(
            body=Src0 * C0 + (Src1 > C1),
            reference=lambda in0, in1, s0, s1, imm2: (
                in0 * np.float32(s0) + (in1 > s1)
            ).astype(np.float32),
        ),
    )
    return m1, oi


_nc_cache: dict = {}


def _build(inv_tau: float):
    """Trace + compile the per-core Bass program (SPMD: same NEFF, 8 cores)."""
    key = float(inv_tau)
    if key in _nc_cache:
        return _nc_cache[key]

    lif_m1, lif_oi = _lif_ops()
    f32 = mybir.dt.float32
    fp16 = mybir.dt.float16
    fp8 = mybir.dt.float8e4

    nc = bacc.Bacc(
        "TRN2",
        target_bir_lowering=False,
        debug=False,
        enable_asserts=False,
        num_devices=CORES,
    )
    # Host pre-transposes each core's shard to [(b j) = 128, T, F] contiguous,
    # so every DMA is a 3-dim AP with a contiguous run per partition.
    x_r = nc.dram_tensor("x", [P, T, F], f32, kind="ExternalInput").ap()
    # Spikes are exactly {0.0, 1.0}: store as fp8-e4m3 (lossless) to cut the
    # HBM write traffic 4x; the host upcasts to f32.
    spk_r = nc.dram_tensor("spk", [P, T, F], fp8, kind="ExternalOutput").ap()
    # oi leaves the chip as fp16 (~2^-10 rel err, see module docstring).
    oi_r = nc.dram_tensor("oi", [P, T, F], fp16, kind="ExternalOutput").ap()

    assert sum(GROUPS) == T
    with TileContext(nc) as tc:
        with (
            tc.tile_pool(name="const", bufs=1) as cpool,
            tc.tile_pool(name="xin", bufs=3) as xpool,
            tc.tile_pool(name="sout", bufs=2) as spool,
            tc.tile_pool(name="state", bufs=1) as mpool,
        ):
            # Spike via one ACT op: sigmoid(S*m1 - (S + 64)) with S = 1e9.
            # fl(S*m1) quantizes to a 64-ulp grid around S, so the argument is
            # always <= -64 (no spike, incl. m1 == V_TH exactly -> -64) or
            # >= +64 (spike); sigmoid saturates to 0.0 / 1.0 there.
            spk_bias = cpool.tile([P, 1], f32)
            nc.gpsimd.memset(spk_bias[:], -(SPK_SCALE + 64.0))

            # Linear state buffers [P, T+1, F]; slot 0 is the zero initial
            # state, step t lives at slot t+1. One DVE op then advances a
            # whole group of M timesteps: out = slots [t+1, t+M+1], state
            # input = slots [t, t+M] — the stream beyond the first F elements
            # reads values the same instruction wrote F elements (~256 DVE
            # cycles, >> pipeline depth) earlier.
            m_buf = mpool.tile([P, T + 1, F], f32, name="m_buf")
            oi_buf = mpool.tile([P, T + 1, F], fp16, name="oi_buf")
            nc.gpsimd.memset(m_buf[:, 0, :], 0.0)
            nc.gpsimd.memset(oi_buf[:, 0, :], 0.0)

            # X streams through a 3-deep rotating pool of group-sized tiles
            # (SBUF can't hold X plus both state buffers). All input DMAs are
            # issued up front on the SP HWDGE ring; pool WAR dependencies
            # self-pace them ~3 groups ahead of compute, and the SDMA engines
            # round-robin packets between this ring and the ACT output ring.
            x_tiles = []
            t0 = 0
            for gi, gl in enumerate(GROUPS):
                xt = xpool.tile([P, max(GROUPS), F], f32, name="x_in")
                x_tiles.append(xt)
                nc.sync.dma_start(
                    out=xt[:, :gl, :], in_=x_r[:, t0 : t0 + gl, :]
                )
                t0 += gl

            t0 = 0
            for gi, gl in enumerate(GROUPS):
                xt = x_tiles[gi]
                # DVE: m1 group — prev = reset(m1_{t-1}); m1_t = prev +
                # (x_t - prev)/TAU, M timesteps per instruction.
                nc.vector._custom_dve(
                    lif_m1,
                    out=m_buf[:, t0 + 1 : t0 + 1 + gl, :],
                    in0=xt[:, :gl, :],
                    in1=m_buf[:, t0 : t0 + gl, :],
                    s0=0.5,      # 1/TAU
                    s1=1.0,      # V_TH
                )
                # DVE: oi group — oi_t = oi_{t-1}*inv_tau + (m1_t > 1),
                # fp16 state in/out, f32 m1 in.
                nc.vector._custom_dve(
                    lif_oi,
                    out=oi_buf[:, t0 + 1 : t0 + 1 + gl, :],
                    in0=oi_buf[:, t0 : t0 + gl, :],
                    in1=m_buf[:, t0 + 1 : t0 + 1 + gl, :],
                    s0=inv_tau,
                    s1=1.0,
                )
                # ACT: spike = sigmoid(S*m1 - (S+64)) in {0, 1}, fp8 out.
                s_t = spool.tile([P, max(GROUPS), F], fp8, name="s_out")
                nc.scalar.activation(
                    out=s_t[:, :gl, :],
                    in_=m_buf[:, t0 + 1 : t0 + 1 + gl, :],
                    func=mybir.ActivationFunctionType.Sigmoid,
                    bias=spk_bias[:],
                    scale=SPK_SCALE,
                )
                # Outputs on the ACT ring, issued as each group completes.
                nc.scalar.dma_start(
                    out=spk_r[:, t0 : t0 + gl, :], in_=s_t[:, :gl, :]
                )
                nc.scalar.dma_start(
                    out=oi_r[:, t0 : t0 + gl, :],
                    in_=oi_buf[:, t0 + 1 : t0 + 1 + gl, :],
                )
                t0 += gl

    nc.compile()
    _nc_cache[key] = nc
    return nc


def _shard(X: np.ndarray) -> list[np.ndarray]:
    """[B, T, N] -> per-core [(b j) = 128, T, F] contiguous."""
    Xt = np.ascontiguousarray(
        X.reshape(B, T, J, F).transpose(0, 2, 1, 3)
    )  # [B, J, T, F]
    return [
        Xt[c * BS : (c + 1) * BS].reshape(P, T, F) for c in range(CORES)
    ]


def _unshard(parts: list[np.ndarray]) -> np.ndarray:
    """per-core [(b j), T, F] -> [B, T, N]."""
    full = np.stack(parts).reshape(B, J, T, F)
    return np.ascontiguousarray(full.transpose(0, 2, 1, 3)).reshape(B, T, N)


def _run(X: np.ndarray, w: np.ndarray, **spmd_kwargs):
    X = np.asarray(X, dtype=np.float32)
    inv_tau = float(1.0 / (1.0 + np.exp(-np.float64(np.asarray(w).item()))))
    nc = _build(inv_tau)
    in_maps = [{"x": xs} for xs in _shard(X)]
    res = run_bass_kernel_spmd(nc, in_maps, core_ids=list(range(CORES)), **spmd_kwargs)
    spikes = _unshard(
        [np.asarray(res.results[c]["spk"]).astype(np.float32) for c in range(CORES)]
    )
    i_pot = _unshard(
        [np.asarray(res.results[c]["oi"]).astype(np.float32) for c in range(CORES)]
    )
    return (spikes, i_pot), res


def kernel(X: np.ndarray, w: np.ndarray):
    out, _ = _run(X, w)
    return out



# revision 11
# speedup vs baseline: 1.0631x; 1.0631x over previous
"""AxonLIFNode forward on 8 Trainium2 NeuronCores.

Reference recurrence (per element, sequential over T):
    mem   = mem + (x_t + V_RESET - mem) / TAU        # V_RESET=0, TAU=2
    spike = (mem - V_TH > 0)                         # V_TH=1, {0.0, 1.0}
    mem   = (1 - spike) * mem + V_RESET * spike      # reset to 0 on spike
    out_i = out_i * sigmoid(w) + spike               # axon current (w=0 -> 0.5)
    outputs: (spike, out_i), both [B, T, N] f32

Strategy: data-parallel over the batch axis (B=64 -> 8 per core). Per core the
32768 independent series are laid out as 128 partitions x 256 free elements;
both recurrences live in linear [P, T+1, F] SBUF buffers (slot 0 = zero init)
and advance MULTIPLE timesteps per DVE instruction via within-instruction
RAW: the op's out stream writes slot t+1 while its state input reads slot t,
i.e. the same SBUF address the instruction itself wrote F=256 elements (~256
DVE cycles, far beyond pipeline depth) earlier. Per group of M timesteps:

    m1 : prev = m1 * (m1 <= 1); m1' = prev + (x - prev) * 0.5   (f32, exact)
    oi : oi' = oi * inv_tau + (m1' > 1)                         (fp16 state)

m1 is bit-exact vs. the reference ordering (each ALU stage is one IEEE f32
rounding; *0.5 == /2 exactly). oi is stored and streamed out as fp16: the
per-step rounding error decays by inv_tau so the accumulated relative error
stays ~2^-10, far inside the harness gate, and the HBM write traffic halves.
Spikes are produced off the critical path on the Scalar(ACT) engine with a
saturated sigmoid (exact {0,1}, see _build) and stored as fp8 (lossless).
X streams in on the SP HWDGE ring (issued up front, batch per group); spike
and oi groups stream out on the ACT ring as each group completes. Group
sizes taper ([4,4,16,...]) so compute starts early and the tail drains small.
"""

import numpy as np

import concourse.bacc as bacc
import concourse.mybir as mybir
import concourse.dve_ops as dve_ops
from concourse.dve_ops import DveOp
from concourse.dve_spec import Spec, Src0, Src1, C0, C1, lower
from concourse.dve_uop import DveOpSpec
from concourse.tile import TileContext
from concourse.bass_utils import run_bass_kernel_spmd

# Problem shape (hardcoded per harness contract).
B, T, N = 64, 64, 4096
CORES = 8
BS = B // CORES          # batches per core
P = 128                  # SBUF partitions
J = 16                   # n-chunks per batch: BS * J == P
F = N // J               # free elements per partition per timestep (256)
GROUPS = [2, 2, 4, 8, 16, 16, 12, 2, 2]  # timesteps per DVE instruction
IN_BATCHES = [2, 2, 4, 8, 16, 16, 16]    # timesteps per input DMA transfer
SPK_SCALE = 1.0e9        # sigmoid saturation trick scale (see _build)

def _register_op(name: str, spec: Spec) -> DveOp:
    """Register a custom DVE op in the global registry with a computed sha."""
    for op in dve_ops.OPS:
        if op.name == name:
            return op
    row = dve_ops._CUSTOM_DVE_ROW_BASE + len(dve_ops.OPS)
    assert row < 0x20, "custom-DVE opcode rows exhausted"
    shas = {}
    for ver in ("v3", "v4"):
        uops = lower(spec, ver=ver)
        shas[ver] = DveOpSpec(name=name, opcode=row, uops=uops, rd1_en=True).sha(ver)
    op = DveOp(name, spec, subdim=False, uops_sha=shas)
    dve_ops._SUB_OPCODE_FOR_NAME[name] = row
    dve_ops.OPS.append(op)
    dve_ops.CUSTOM_DVE_SPECS[name] = spec
    return op


def _lif_ops() -> tuple[DveOp, DveOp]:
    """LIF_M1: m1_t from (x_t, m1_{t-1}); LIF_OI: oi_t from (oi_{t-1}, m1_t).

    LIF_M1: out = prev + (Src0 - prev) * C0, prev = Src1 * (Src1 <= C1)
    LIF_OI: out = Src0 * C0 + (Src1 > C1)
    Each ALU stage is one IEEE f32 rounding; bit-exact vs the reference.
    """
    keep = Src1 <= C1
    prev = Src1 * keep
    m1 = _register_op(
        "LIF_M1_ANT",
        Spec(
            body=prev + (Src0 - prev) * C0,
            reference=lambda in0, in1, s0, s1, imm2: (
                (p := (in1 * (in1 <= s1)).astype(np.float32))
                + (in0 - p) * np.float32(s0)
            ).astype(np.float32),
        ),
    )
    oi = _register_op(
        "LIF_OI_ANT",
        Spec(
            body=Src0 * C0 + (Src1 > C1),
            reference=lambda in0, in1, s0, s1, imm2: (
                in0 * np.float32(s0) + (in1 > s1)
            ).astype(np.float32),
        ),
    )
    return m1, oi


_nc_cache: dict = {}


def _build(inv_tau: float):
    """Trace + compile the per-core Bass program (SPMD: same NEFF, 8 cores)."""
    key = float(inv_tau)
    if key in _nc_cache:
        return _nc_cache[key]

    lif_m1, lif_oi = _lif_ops()
    f32 = mybir.dt.float32
    fp16 = mybir.dt.float16
    fp8 = mybir.dt.float8e4

    nc = bacc.Bacc(
        "TRN2",
        target_bir_lowering=False,
        debug=False,
        enable_asserts=False,
        num_devices=CORES,
    )
    # Host pre-transposes each core's shard to [(b j) = 128, T, F] contiguous,
    # so every DMA is a 3-dim AP with a contiguous run per partition.
    x_r = nc.dram_tensor("x", [P, T, F], f32, kind="ExternalInput").ap()
    # Spikes are exactly {0.0, 1.0}: store as fp8-e4m3 (lossless) to cut the
    # HBM write traffic 4x; the host upcasts to f32.
    spk_r = nc.dram_tensor("spk", [P, T, F], fp8, kind="ExternalOutput").ap()
    # oi leaves the chip as fp16 (~2^-10 rel err, see module docstring).
    oi_r = nc.dram_tensor("oi", [P, T, F], fp16, kind="ExternalOutput").ap()

    assert sum(GROUPS) == T and sum(IN_BATCHES) == T
    # every compute group must lie inside one input batch
    bounds = set(np.cumsum([0] + IN_BATCHES).tolist())
    acc = 0
    for gl in GROUPS:
        assert any(b <= acc < acc + gl <= b2 for b in bounds for b2 in bounds if b2 > b)
        acc += gl
    with TileContext(nc) as tc:
        with (
            tc.tile_pool(name="const", bufs=1) as cpool,
            tc.tile_pool(name="sout", bufs=4) as spool,
            tc.tile_pool(name="state", bufs=1) as mpool,
        ):
            # Spike via one ACT op: sigmoid(S*m1 - (S + 64)) with S = 1e9.
            # fl(S*m1) quantizes to a 64-ulp grid around S, so the argument is
            # always <= -64 (no spike, incl. m1 == V_TH exactly -> -64) or
            # >= +64 (spike); sigmoid saturates to 0.0 / 1.0 there.
            spk_bias = cpool.tile([P, 1], f32)
            nc.vector.memset(spk_bias[:], -(SPK_SCALE + 64.0))

            # Linear state buffers [P, T+1, F]; slot 0 is the zero initial
            # state, step t lives at slot t+1. One DVE op advances a whole
            # group of M timesteps: out = slots [t+1, t+M+1], state input =
            # slots [t, t+M] — the stream beyond the first F elements reads
            # values the same instruction wrote F elements (~256 DVE cycles,
            # >> pipeline depth) earlier. X is DMA'd INTO xm slots 1..T and
            # the m1 op runs IN PLACE (in0 == out: each x is read ~8 cycles
            # before m1 overwrites it — verified bit-exact on HW), so X needs
            # no separate staging and all input DMAs go out up front, ungated.
            xm = mpool.tile([P, T + 1, F], f32, name="xm")
            oi_buf = mpool.tile([P, T + 1, F], fp16, name="oi_buf")
            nc.vector.memset(xm[:, 0, :], 0.0)
            nc.vector.memset(oi_buf[:, 0, :], 0.0)

            t0 = 0
            for bl in IN_BATCHES:
                nc.sync.dma_start(
                    out=xm[:, t0 + 1 : t0 + 1 + bl, :],
                    in_=x_r[:, t0 : t0 + bl, :],
                )
                t0 += bl

            t0 = 0
            for gi, gl in enumerate(GROUPS):
                # DVE: m1 group — prev = reset(m1_{t-1}); m1_t = prev +
                # (x_t - prev)/TAU, in place over the x slots.
                nc.vector._custom_dve(
                    lif_m1,
                    out=xm[:, t0 + 1 : t0 + 1 + gl, :],
                    in0=xm[:, t0 + 1 : t0 + 1 + gl, :],
                    in1=xm[:, t0 : t0 + gl, :],
                    s0=0.5,      # 1/TAU
                    s1=1.0,      # V_TH
                )
                # DVE: oi group — oi_t = oi_{t-1}*inv_tau + (m1_t > 1),
                # fp16 state in/out, f32 m1 in.
                nc.vector._custom_dve(
                    lif_oi,
                    out=oi_buf[:, t0 + 1 : t0 + 1 + gl, :],
                    in0=oi_buf[:, t0 : t0 + gl, :],
                    in1=xm[:, t0 + 1 : t0 + 1 + gl, :],
                    s0=inv_tau,
                    s1=1.0,
                )
                # ACT: spike = sigmoid(S*m1 - (S+64)) in {0, 1}, fp8 out.
                s_t = spool.tile([P, max(GROUPS), F], fp8, name="s_out")
                nc.scalar.activation(
                    out=s_t[:, :gl, :],
                    in_=xm[:, t0 + 1 : t0 + 1 + gl, :],
                    func=mybir.ActivationFunctionType.Sigmoid,
                    bias=spk_bias[:],
                    scale=SPK_SCALE,
                )
                # Outputs stream as each group completes: spikes on the ACT
                # ring; oi on the ACT ring early, on the (by then idle) SP
                # ring for the tail groups so the final drain uses both rings.
                nc.scalar.dma_start(
                    out=spk_r[:, t0 : t0 + gl, :], in_=s_t[:, :gl, :]
                )
                oi_eng = nc.sync if t0 >= 48 else nc.scalar
                oi_eng.dma_start(
                    out=oi_r[:, t0 : t0 + gl, :],
                    in_=oi_buf[:, t0 + 1 : t0 + 1 + gl, :],
                )
                t0 += gl

    nc.compile()
    _nc_cache[key] = nc
    return nc


def _shard(X: np.ndarray) -> list[np.ndarray]:
    """[B, T, N] -> per-core [(b j) = 128, T, F] contiguous."""
    Xt = np.ascontiguousarray(
        X.reshape(B, T, J, F).transpose(0, 2, 1, 3)
    )  # [B, J, T, F]
    return [
        Xt[c * BS : (c + 1) * BS].reshape(P, T, F) for c in range(CORES)
    ]


def _unshard(parts: list[np.ndarray]) -> np.ndarray:
    """per-core [(b j), T, F] -> [B, T, N]."""
    full = np.stack(parts).reshape(B, J, T, F)
    return np.ascontiguousarray(full.transpose(0, 2, 1, 3)).reshape(B, T, N)


def _run(X: np.ndarray, w: np.ndarray, **spmd_kwargs):
    X = np.asarray(X, dtype=np.float32)
    inv_tau = float(1.0 / (1.0 + np.exp(-np.float64(np.asarray(w).item()))))
    nc = _build(inv_tau)
    in_maps = [{"x": xs} for xs in _shard(X)]
    res = run_bass_kernel_spmd(nc, in_maps, core_ids=list(range(CORES)), **spmd_kwargs)
    spikes = _unshard(
        [np.asarray(res.results[c]["spk"]).astype(np.float32) for c in range(CORES)]
    )
    i_pot = _unshard(
        [np.asarray(res.results[c]["oi"]).astype(np.float32) for c in range(CORES)]
    )
    return (spikes, i_pot), res


def kernel(X: np.ndarray, w: np.ndarray):
    out, _ = _run(X, w)
    return out



# revision 18
# speedup vs baseline: 1.1693x; 1.0999x over previous
"""AxonLIFNode forward on 8 Trainium2 NeuronCores.

Reference recurrence (per element, sequential over T):
    mem   = mem + (x_t + V_RESET - mem) / TAU        # V_RESET=0, TAU=2
    spike = (mem - V_TH > 0)                         # V_TH=1, {0.0, 1.0}
    mem   = (1 - spike) * mem + V_RESET * spike      # reset to 0 on spike
    out_i = out_i * sigmoid(w) + spike               # axon current (w=0 -> 0.5)
    outputs: (spike, out_i), both [B, T, N] f32

Strategy: data-parallel over the batch axis (B=64 -> 8 per core). Per core the
32768 independent series are laid out as 128 partitions x 256 free elements;
both recurrences live in linear [P, T+1, F] SBUF buffers (slot 0 = zero init)
and advance MULTIPLE timesteps per DVE instruction via within-instruction
RAW: the op's out stream writes slot t+1 while its state input reads slot t,
i.e. the same SBUF address the instruction itself wrote F=256 elements (~256
DVE cycles, far beyond pipeline depth) earlier. Per group of M timesteps:

    m1 : prev = m1 * (m1 <= 1); m1' = prev + (x - prev) * 0.5   (f32, exact)
    oi : oi' = oi * inv_tau + (m1' > 1)                         (fp16 state)

m1 is bit-exact vs. the reference ordering (each ALU stage is one IEEE f32
rounding; *0.5 == /2 exactly). oi is stored and streamed out as fp16: the
per-step rounding error decays by inv_tau so the accumulated relative error
stays ~2^-10, far inside the harness gate, and the HBM write traffic halves.
Spikes are produced off the critical path on the Scalar(ACT) engine with a
saturated sigmoid (exact {0,1}, see _build) and stored as fp8 (lossless).
X streams in on the SP HWDGE ring (issued up front, batch per group); spike
and oi groups stream out on the ACT ring as each group completes. Group
sizes taper ([4,4,16,...]) so compute starts early and the tail drains small.
"""

import numpy as np

import concourse.bacc as bacc
import concourse.mybir as mybir
import concourse.dve_ops as dve_ops
from concourse.dve_ops import DveOp
from concourse.dve_spec import Spec, Src0, Src1, C0, C1, lower
from concourse.dve_uop import (
    AluInp,
    AluOp,
    DelayInp,
    DveOpSpec,
    InpSel,
    OutPath,
    OutSel,
    Trigger,
    UopConfig,
    UopDpConfig,
)
from concourse.tile import TileContext
from concourse.bass_utils import run_bass_kernel_spmd

# Problem shape (hardcoded per harness contract).
B, T, N = 64, 64, 4096
CORES = 8
BS = B // CORES          # batches per core
P = 128                  # SBUF partitions
J = 16                   # n-chunks per batch: BS * J == P
F = N // J               # free elements per partition per timestep (256)
GROUPS = [2, 2, 4, 8, 16, 16, 8, 4, 2, 2]  # timesteps per DVE instruction
IN_BATCHES = [2, 2, 4, 8, 16, 16, 16]      # timesteps per input DMA transfer
SPK_SCALE = 1.0e9        # sigmoid saturation trick scale (see _build)

def _register_op(name: str, spec: Spec) -> DveOp:
    """Register a custom DVE op in the global registry with a computed sha."""
    for op in dve_ops.OPS:
        if op.name == name:
            return op
    row = dve_ops._CUSTOM_DVE_ROW_BASE + len(dve_ops.OPS)
    assert row < 0x20, "custom-DVE opcode rows exhausted"
    shas = {}
    for ver in ("v3", "v4"):
        uops = lower(spec, ver=ver)
        shas[ver] = DveOpSpec(name=name, opcode=row, uops=uops, rd1_en=True).sha(ver)
    op = DveOp(name, spec, subdim=False, uops_sha=shas)
    dve_ops._SUB_OPCODE_FOR_NAME[name] = row
    dve_ops.OPS.append(op)
    dve_ops.CUSTOM_DVE_SPECS[name] = spec
    return op


def _oi_pk_uops_2x() -> list[UopConfig]:
    """Hand-authored 2X_1PORT program for `out = Src0*C0 + Src1`.

    Mirrors the stock TENSOR_MASK 2x row (slot 105): input lanes carry the
    LO halves plus SRC_*_HI; stages 0-1 compute the LO element, stage 2
    captures it into delay lane 0 while computing the HI product, stage 3
    finishes the HI element, bypass stages carry it to the write mux, and
    the pair re-packs as WR0_LO <- DELAY_0 / WR0_HI <- ALU_OUT.
    """
    # inp[0] feeds the pipe head (block -1 "PREV_ALU_OUT"); delay lanes
    # D0..D5 are inp[1..6] (decoded from the stock TENSOR_MASK 2x row).
    lanes = [
        InpSel.ZERO,      # pipe head (unused)
        InpSel.SRC_0,     # D0
        InpSel.CONST_0,   # D1
        InpSel.SRC_1,     # D2
        InpSel.SRC_0_HI,  # D3
        InpSel.SRC_1_HI,  # D4
        InpSel.ZERO,
        InpSel.ZERO,
    ]
    D = (
        AluInp.PREV_DELAY_0,
        AluInp.PREV_DELAY_1,
        AluInp.PREV_DELAY_2,
        AluInp.PREV_DELAY_3,
        AluInp.PREV_DELAY_4,
    )
    dp = [UopDpConfig() for _ in range(8)]
    # s0: p_lo = SRC_0 * C0; carry C0, SRC_1, SRC_0_HI, SRC_1_HI
    dp[0].enable_alu(AluOp.MULTIPLY, D[0], D[1])
    dp[0].pass_through_delay(1, 2, 3, 4)
    # s1: r_lo = p_lo + SRC_1; carry C0, SRC_0_HI, SRC_1_HI
    dp[1].enable_alu(AluOp.ADD, AluInp.PREV_ALU_OUT, D[2])
    dp[1].pass_through_delay(1, 3, 4)
    # s2: p_hi = SRC_0_HI * C0; D0 captures r_lo; carry SRC_1_HI
    dp[2].enable_alu(AluOp.MULTIPLY, D[3], D[1])
    dp[2].enable_delay_from_src(DelayInp.PREV_ALU_OUT, 0)
    dp[2].pass_through_delay(4)
    # s3: r_hi = p_hi + SRC_1_HI; carry r_lo in D0
    dp[3].enable_alu(AluOp.ADD, AluInp.PREV_ALU_OUT, D[4])
    dp[3].pass_through_delay(0)
    # s4-7: bypass r_hi down the ALU chain; D0 carries r_lo
    for k in range(4, 8):
        dp[k].pass_through_alu()
        dp[k].pass_through_delay(0)
    return [
        UopConfig(
            inp=lanes,
            inp_enable=[0, 1, 1, 1, 1, 1, 0, 0],
            out={
                OutPath.WR0_LO: OutSel.DELAY_0,
                OutPath.WR0_HI: OutSel.ALU_OUT,
                OutPath.WR1_LO: OutSel.ALU_OUT,
                OutPath.WR1_HI: OutSel.ALU_OUT,
            },
            out_enable={
                OutPath.WR0_LO: 1,
                OutPath.WR0_HI: 1,
                OutPath.WR1_LO: 0,
                OutPath.WR1_HI: 0,
            },
            require_inp0=1,
            require_inp1=1,
            trigger=(Trigger.SRC_TENSOR_DONE, Trigger.NONE, Trigger.NONE),
            datapath_config=dp,
        )
    ]


def _register_op_2x(name: str, spec: Spec) -> DveOp:
    """Register a custom DVE op that also carries a hand-authored 2X_1PORT
    uop program (engaged when the emitting site sets inst.perf_max=1 and all
    tensor APs are 16-bit/contiguous; the engine silently falls back to the
    1x program otherwise). Injected via the compile cache, which both
    `_custom_dve` and `dve_table_for_ops` consult before re-lowering."""
    for op in dve_ops.OPS:
        if op.name == name:
            return op
    row = dve_ops._CUSTOM_DVE_ROW_BASE + len(dve_ops.OPS)
    assert row < 0x20, "custom-DVE opcode rows exhausted"
    shas = {}
    for ver in ("v3", "v4"):
        s = DveOpSpec(
            name=name,
            opcode=row,
            uops=lower(spec, ver=ver),
            rd1_en=True,
            uops_2x=_oi_pk_uops_2x(),
            perf_max=1,
        )
        s.validate(ver)
        shas[ver] = s.sha(ver)
        dve_ops._COMPILE_CACHE[(name, ver)] = s
    op = DveOp(name, spec, subdim=False, uops_sha=shas)
    dve_ops._SUB_OPCODE_FOR_NAME[name] = row
    dve_ops.OPS.append(op)
    dve_ops.CUSTOM_DVE_SPECS[name] = spec
    return op


def _lif_ops() -> tuple[DveOp, DveOp]:
    """LIF_M1: m1_t from (x_t, m1_{t-1}); LIF_OI_PK: oi_t from
    (oi_{t-1}, spk_t).

    LIF_M1:    out = prev + (Src0 - prev) * C0, prev = Src1 * (Src1 <= C1)
    LIF_OI_PK: out = Src0 * C0 + Src1   (Src1 = spike, already {0,1} exact)
    Each ALU stage is one IEEE f32 rounding; m1 is bit-exact vs the
    reference. LIF_OI_PK carries a 2X_1PORT uop variant (2 elem/cycle when
    all operands are fp16).
    """
    keep = Src1 <= C1
    prev = Src1 * keep
    m1 = _register_op(
        "LIF_M1_ANT",
        Spec(
            body=prev + (Src0 - prev) * C0,
            reference=lambda in0, in1, s0, s1, imm2: (
                (p := (in1 * (in1 <= s1)).astype(np.float32))
                + (in0 - p) * np.float32(s0)
            ).astype(np.float32),
        ),
    )
    oi = _register_op_2x(
        "LIF_OI_PK_ANT",
        Spec(
            body=Src0 * C0 + Src1,
            reference=lambda in0, in1, s0, s1, imm2: (
                in0.astype(np.float32) * np.float32(s0)
                + in1.astype(np.float32)
            ).astype(np.float32),
        ),
    )
    return m1, oi


_nc_cache: dict = {}


def _build(inv_tau: float):
    """Trace + compile the per-core Bass program (SPMD: same NEFF, 8 cores)."""
    key = float(inv_tau)
    if key in _nc_cache:
        return _nc_cache[key]

    lif_m1, lif_oi = _lif_ops()
    f32 = mybir.dt.float32
    fp16 = mybir.dt.float16
    fp8 = mybir.dt.float8e4

    nc = bacc.Bacc(
        "TRN2",
        target_bir_lowering=False,
        debug=False,
        enable_asserts=False,
        num_devices=CORES,
    )
    # Host pre-transposes each core's shard to [(b j) = 128, T, F] contiguous,
    # so every DMA is a 3-dim AP with a contiguous run per partition.
    x_r = nc.dram_tensor("x", [P, T, F], f32, kind="ExternalInput").ap()
    # Spikes are exactly {0.0, 1.0}: store as fp8-e4m3 (lossless) to cut the
    # HBM write traffic 4x; the host upcasts to f32.
    spk_r = nc.dram_tensor("spk", [P, T, F], fp8, kind="ExternalOutput").ap()
    # oi leaves the chip as fp16 (~2^-10 rel err, see module docstring).
    oi_r = nc.dram_tensor("oi", [P, T, F], fp16, kind="ExternalOutput").ap()

    assert sum(GROUPS) == T and sum(IN_BATCHES) == T
    # every compute group must lie inside one input batch
    bounds = set(np.cumsum([0] + IN_BATCHES).tolist())
    acc = 0
    for gl in GROUPS:
        assert any(b <= acc < acc + gl <= b2 for b in bounds for b2 in bounds if b2 > b)
        acc += gl
    with TileContext(nc) as tc:
        with (
            tc.tile_pool(name="const", bufs=1) as cpool,
            tc.tile_pool(name="sout", bufs=4) as spool,
            tc.tile_pool(name="state", bufs=1) as mpool,
        ):
            # Spike via one ACT op: sigmoid(S*m1 - (S + 64)) with S = 1e9.
            # fl(S*m1) quantizes to a 64-ulp grid around S, so the argument is
            # always <= -64 (no spike, incl. m1 == V_TH exactly -> -64) or
            # >= +64 (spike); sigmoid saturates to 0.0 / 1.0 there.
            spk_bias = cpool.tile([P, 1], f32)
            nc.vector.memset(spk_bias[:], -(SPK_SCALE + 64.0))

            # Linear state buffers [P, T+1, F]; slot 0 is the zero initial
            # state, step t lives at slot t+1. One DVE op advances a whole
            # group of M timesteps: out = slots [t+1, t+M+1], state input =
            # slots [t, t+M] — the stream beyond the first F elements reads
            # values the same instruction wrote F elements (~256 DVE cycles,
            # >> pipeline depth) earlier. X is DMA'd INTO xm slots 1..T and
            # the m1 op runs IN PLACE (in0 == out: each x is read ~8 cycles
            # before m1 overwrites it — verified bit-exact on HW), so X needs
            # no separate staging and all input DMAs go out up front, ungated.
            xm = mpool.tile([P, T + 1, F], f32, name="xm")
            oi_buf = mpool.tile([P, T + 1, F], fp16, name="oi_buf")
            nc.vector.memset(xm[:, 0, :], 0.0)
            nc.vector.memset(oi_buf[:, 0, :], 0.0)

            t0 = 0
            for bl in IN_BATCHES:
                nc.sync.dma_start(
                    out=xm[:, t0 + 1 : t0 + 1 + bl, :],
                    in_=x_r[:, t0 : t0 + bl, :],
                )
                t0 += bl

            # Pipeline with oi lagging m1 by one group: DVE order is
            # m1(0), m1(1), oi(0), m1(2), oi(1), ..., m1(9), oi(8), oi(9).
            # oi(g) consumes the fp16 spikes sig(g) produced on ACT while
            # DVE ran m1(g+1), so neither engine waits on the other. The
            # fp8 HBM spike copy is a casting SWDGE store on the otherwise
            # idle GPSIMD ring; oi stores go on the SP ring (plain issues
            # that never block compute), leaving the ACT queue pure compute.
            starts = np.cumsum([0] + GROUPS).tolist()
            s_tiles = {}

            def emit_m1(g):
                t0, gl = starts[g], GROUPS[g]
                nc.vector._custom_dve(
                    lif_m1,
                    out=xm[:, t0 + 1 : t0 + 1 + gl, :],
                    in0=xm[:, t0 + 1 : t0 + 1 + gl, :],
                    in1=xm[:, t0 : t0 + gl, :],
                    s0=0.5,      # 1/TAU
                    s1=1.0,      # V_TH
                )
                # ACT: spike = sigmoid(S*m1 - (S+64)) in {0, 1}, fp16 out.
                s_t = spool.tile([P, max(GROUPS), F], fp16, name="s_out")
                s_tiles[g] = s_t
                nc.scalar.activation(
                    out=s_t[:, :gl, :],
                    in_=xm[:, t0 + 1 : t0 + 1 + gl, :],
                    func=mybir.ActivationFunctionType.Sigmoid,
                    bias=spk_bias[:],
                    scale=SPK_SCALE,
                )
                # fp16 -> fp8 cast during the store (SWDGE-only feature).
                nc.gpsimd.dma_start(
                    out=spk_r[:, t0 : t0 + gl, :], in_=s_t[:, :gl, :]
                )

            def emit_oi(g):
                t0, gl = starts[g], GROUPS[g]
                nc.vector._custom_dve(
                    lif_oi,
                    out=oi_buf[:, t0 + 1 : t0 + 1 + gl, :],
                    in0=oi_buf[:, t0 : t0 + gl, :],
                    in1=s_tiles[g][:, :gl, :],
                    s0=inv_tau,
                    s1=0.0,
                )
                nc.sync.dma_start(
                    out=oi_r[:, t0 : t0 + gl, :],
                    in_=oi_buf[:, t0 + 1 : t0 + 1 + gl, :],
                )

            n = len(GROUPS)
            emit_m1(0)
            for g in range(1, n):
                emit_m1(g)
                emit_oi(g - 1)
            emit_oi(n - 1)

    # Tile's scheduler re-emits instructions, so stamp the 2x reachability
    # bit (ISA byte-36[7:6]) on the final module: the engine then picks the
    # 2X_1PORT uop slot when the APs qualify (all-fp16, stride 1, aligned).
    for fn in nc.m.functions:
        for bb in fn.blocks:
            for inst in bb.instructions:
                if (
                    type(inst).__name__ == "InstCustomDveAnt"
                    and inst.op_name == "LIF_OI_PK_ANT"
                ):
                    inst.perf_max = 1

    nc.compile()
    _nc_cache[key] = nc
    return nc


def _shard(X: np.ndarray) -> list[np.ndarray]:
    """[B, T, N] -> per-core [(b j) = 128, T, F] contiguous."""
    Xt = np.ascontiguousarray(
        X.reshape(B, T, J, F).transpose(0, 2, 1, 3)
    )  # [B, J, T, F]
    return [
        Xt[c * BS : (c + 1) * BS].reshape(P, T, F) for c in range(CORES)
    ]


def _unshard(parts: list[np.ndarray]) -> np.ndarray:
    """per-core [(b j), T, F] -> [B, T, N]."""
    full = np.stack(parts).reshape(B, J, T, F)
    return np.ascontiguousarray(full.transpose(0, 2, 1, 3)).reshape(B, T, N)


def _run(X: np.ndarray, w: np.ndarray, **spmd_kwargs):
    X = np.asarray(X, dtype=np.float32)
    inv_tau = float(1.0 / (1.0 + np.exp(-np.float64(np.asarray(w).item()))))
    nc = _build(inv_tau)
    in_maps = [{"x": xs} for xs in _shard(X)]
    res = run_bass_kernel_spmd(nc, in_maps, core_ids=list(range(CORES)), **spmd_kwargs)
    spikes = _unshard(
        [np.asarray(res.results[c]["spk"]).astype(np.float32) for c in range(CORES)]
    )
    i_pot = _unshard(
        [np.asarray(res.results[c]["oi"]).astype(np.float32) for c in range(CORES)]
    )
    return (spikes, i_pot), res


def kernel(X: np.ndarray, w: np.ndarray):
    out, _ = _run(X, w)
    return out

